# revision 3
# baseline (speedup 1.0000x reference)
"""Trainium2 Bass kernel for nn_MaxYager2d — v2 (zero-DMA-hop reshape).

Math (same softmin rewrite as v1):
  out[b,f,h,w] = max_j relu(1 - (a_j + b_jf)^(2/3-ish)) -> softmin via
  P = sum_j exp(-s*a_j) * exp(-s*b_jf), a factorized 3x3 conv on the PE.

v2 structural changes vs v1:
- x staged q-MAJOR: SBUF partition (32q + c) holds quarter q of channel c's
  unfolded row.  The ACT chain (Ln/Exp/Exp) runs at [128, 561] rates.
- The [96, 2246] matmul tile T (rows kh*32+c) is built by UNIFORM
  partition-shift copies on DVE/GPSIMD/ACT (proven on HW) instead of the
  e32+e3 DMA hops -> removes 2x ~2.2us DMA latency from the spine.
- PSUM [128, 561]: all 4 output quarters packed on partitions via
  tile_position so the epilogue runs [128]-wide.
- Output DMA'd as bf16, host upcasts.
"""

import numpy as np

C = 32
K = 3
H = 66
S = 64
B = 4
F = 32
NCORES = 8

RIN = 34                # input rows per core (32 out + 2 halo)
W32 = RIN * H           # 2244 (c-row width)
QW = W32 // 4           # 561 (quarter width)
TW = W32 + 2            # 2246 (T width, incl kw pad)
PIECES = [(0, 256), (256, 512), (512, QW)]  # bank-contained column pieces
OW = 640                # padded output row (1280B, multiple of 256B)

SOFT_S = 400.0
P15 = 1.5
EXP_BIAS = -(2.0 / 3.0) * float(np.log(SOFT_S))
EPS = 1.2e-38

_cache = {}
IN_BF16 = True


def _build_program(warmup=12, prep_chunks=2, in_bf16=IN_BF16):
    import concourse.tile as tile
    from concourse import bacc, mybir

    f32 = mybir.dt.float32
    bf16 = mybir.dt.bfloat16
    Alu = mybir.AluOpType
    Act = mybir.ActivationFunctionType

    nc = bacc.Bacc("TRN2", target_bir_lowering=False, debug=False,
                   num_devices=NCORES)

    in_dt = bf16 if in_bf16 else f32
    x_c = nc.dram_tensor("x_c", [128, QW], in_dt, kind="ExternalInput").ap()
    w_sc = nc.dram_tensor("w_sc", [96, 96], f32, kind="ExternalInput").ap()
    # output written by dma_scatter_add into a zeroed buffer; rows padded to
    # 640 bf16 = 1280B (descriptor element sizes must be 256B multiples)
    out_d = nc.dram_tensor("out", [128, 1, OW], bf16,
                           kind="ExternalOutput").ap()
    idx_d = nc.dram_tensor("idx_d", [128, 8], mybir.dt.int16,
                           kind="ExternalInput").ap()

    with tile.TileContext(nc) as tc:
        with tc.tile_pool(name="sb", bufs=1) as sb, \
             tc.tile_pool(name="ps", bufs=1, space="PSUM") as ps:
            # PSUM: one tile per column piece so the epilogue of piece i only
            # depends on piece i's matmuls.  Tiles pack from a bank-aligned
            # base; each piece's matmul window stays inside one 2KB bank.
            PT = [ps.tile([128, p1 - p0], f32, name=f"pt{i}")
                  for i, (p0, p1) in enumerate(PIECES)]
            pw = ps.tile([128, 512], f32)
            # ---- input DMA first (longest-latency chain), column-chunked
            # (uneven: big c0 so the Ln->Exp chain never stalls on c1) ----
            bounds = [0, 400, QW] if prep_chunks == 2 else \
                [QW * i // prep_chunks for i in range(prep_chunks + 1)]
            xt = sb.tile([128, QW], in_dt)
            for c0, c1 in zip(bounds, bounds[1:]):
                nc.sync.dma_start(xt[:, c0:c1], x_c[:, c0:c1])
            wt = sb.tile([96, 96], f32)
            nc.sync.dma_start(wt[:], w_sc)

            # single ACT table set for Ln/Exp (id 6)
            nc.scalar.add_instruction(mybir.InstLoadActFuncSet(
                name=nc.get_next_instruction_name(), ins=[], outs=[],
                act_func_set_id=6))

            ws = sb.tile([96, 512], bf16)
            if warmup:
                nc.vector.memset(ws[:], 1.0)

            # ---- x prep: EE = exp(-s (1-x)^1.5), q-major [128, 561] ----
            # weight chain emitted first: wLn runs before x arrives and
            # wE1/wE2 fill the x-chain's DMA-wait gaps, so G is ready well
            # before the first conv matmul.
            lg = sb.tile([128, QW], f32)
            vt = sb.tile([128, QW], f32)
            EE = sb.tile([128, QW], bf16)
            lw = sb.tile([96, 96], f32)
            vw = sb.tile([96, 96], f32)
            G = sb.tile([96, 96], bf16)
            # wLn fills the Ln-c0 -> Ln-c1 DMA-wait gap in the x chain
            nc.scalar.activation(lg[:, bounds[0]:bounds[1]],
                                 xt[:, bounds[0]:bounds[1]], Act.Ln,
                                 bias=1.0, scale=-1.0)
            nc.scalar.activation(lw[:], wt[:], Act.Ln, bias=1.0, scale=-1.0)
            for c0, c1 in zip(bounds[1:], bounds[2:]):
                nc.scalar.activation(lg[:, c0:c1], xt[:, c0:c1], Act.Ln,
                                     bias=1.0, scale=-1.0)
            for c0, c1 in zip(bounds, bounds[1:]):
                nc.scalar.activation(vt[:, c0:c1], lg[:, c0:c1], Act.Exp,
                                     scale=P15)
            for c0, c1 in zip(bounds, bounds[1:]):
                nc.scalar.activation(EE[:, c0:c1], vt[:, c0:c1], Act.Exp,
                                     scale=-SOFT_S)
            nc.scalar.activation(vw[:], lw[:], Act.Exp, scale=P15)
            G = G  # noqa
            nc.scalar.activation(G[:], vw[:], Act.Exp, scale=-SOFT_S)

            # epilogue per-partition scalars
            bias_t = sb.tile([128, 1], f32)
            nc.gpsimd.memset(bias_t[:], EXP_BIAS)
            eps_t = sb.tile([128, 1], f32)
            nc.gpsimd.memset(eps_t[:], EPS)

            # scatter-out identity indices [128, 8] int16: idx i at
            # [i % 16 wrapped rows, replicated to all 8 Q7 cores]
            i16 = mybir.dt.int16
            idxs = sb.tile([128, 8], i16)
            nc.sync.dma_start(idxs[:], idx_d)
            ov3 = sb.tile([128, 1, OW], bf16)
            nc.vector.memset(ov3[:, :, QW:OW], 0.0)
            # scatter-add needs a zeroed DRAM target; PJRT does not zero
            # output buffers, so clear it with an early (fully hidden) DMA
            zz = sb.tile([128, 1, OW], bf16)
            nc.vector.memset(zz[:], 0.0)
            nc.sync.dma_start(out_d, zz[:])
            ssem = nc.alloc_semaphore("s_dma")
            shim_sem = nc.alloc_semaphore("shim_dma")
            shim_t = sb.tile([1, 1], f32)
            nc.gpsimd.dma_scatter_add(out_d, ov3[:], idxs[:], 128, 128, OW,
                                      prepare_only=True, sem=ssem)

            # ---- T [96, 2246] rows (kh*32 + c) via partition-shift copies ----
            T = sb.tile([96, TW], bf16)
            nc.gpsimd.memset(T[:, 2112:TW], 1.0)  # junk tail, keeps ln finite

            def copy_op(eng, q, kh, lo=0, hi=QW):
                # copy src-col subrange [lo, hi) of quarter q's kh-shifted row
                d0 = max(0, QW * q - H * kh)
                d1 = min(QW * (q + 1) - H * kh, W32 - H * kh)
                s0 = d0 - QW * q + H * kh
                s1 = s0 + (d1 - d0)
                a, b = max(s0, lo), min(s1, hi)
                if a >= b:
                    return
                dst = T[32 * kh:32 * kh + 32, d0 + (a - s0):d0 + (b - s0)]
                src = EE[32 * q:32 * q + 32, a:b]
                if eng == "v":
                    nc.vector.tensor_scalar(dst, src, 1.0, None, Alu.mult)
                elif eng == "p":
                    nc.gpsimd.tensor_scalar(dst, src, 1.0, None, Alu.mult)
                else:
                    nc.scalar.copy(dst, src)

            # Copies split across engines, emitted in the order matmul
            # groups need them.  GPSIMD takes two slow full-width kh0
            # blocks, ACT takes one after the weight chain.
            for eng, q, kh in (("v", 0, 0), ("v", 0, 1), ("v", 0, 2),
                               ("p", 1, 0), ("v", 1, 1), ("v", 1, 2),
                               ("s", 2, 0), ("v", 2, 1), ("v", 2, 2),
                               ("p", 3, 0), ("v", 3, 1), ("v", 3, 2)):
                copy_op(eng, q, kh)

            # ---- PE warmup then conv matmuls ----
            if warmup:
                for _ in range(warmup):
                    nc.tensor.matmul(pw[:, :], ws[:, 0:128], ws[:, :],
                                     start=True, stop=True)

            def mm(g, pi):
                p0, p1 = PIECES[pi]
                for kw in range(K):
                    nc.tensor.matmul(
                        PT[pi][32 * g:32 * (g + 1), :],
                        G[:, 32 * kw:32 * (kw + 1)],
                        T[:, QW * g + p0 + kw: QW * g + p1 + kw],
                        start=(kw == 0), stop=(kw == 2),
                        tile_position=(0, 32 * g))

            # tiny tail pieces first: the cost model runs the first two
            # matmuls after a PE break at half clock, so burn that on N=49
            mm(0, 2)
            mm(1, 2)
            for g in range(4):
                mm(g, 0)
            mm(2, 2)
            mm(3, 2)
            for g in range(4):
                mm(g, 1)

            # ---- epilogue: out = 1 - ((-ln P)/s)^(2/3), chunked A/B ----
            qv = sb.tile([128, QW], f32)
            q2 = sb.tile([128, QW], f32)
            rv = sb.tile([128, QW], f32)
            # op-major piece interleave: each op's write-ack drain hides
            # under the other pieces' ops
            for pi, (p0, p1) in enumerate(PIECES):
                nc.scalar.activation(qv[:, p0:p1], PT[pi][:, :], Act.Ln,
                                     bias=eps_t[:])
            for p0, p1 in PIECES:
                nc.vector.tensor_scalar(qv[:, p0:p1], qv[:, p0:p1],
                                        -1e-4, None, Alu.min)
            for p0, p1 in PIECES:
                nc.scalar.activation(q2[:, p0:p1], qv[:, p0:p1], Act.Ln,
                                     scale=-1.0)
            for p0, p1 in PIECES:
                nc.scalar.activation(rv[:, p0:p1], q2[:, p0:p1], Act.Exp,
                                     scale=2.0 / 3.0, bias=bias_t[:])
            for p0, p1 in PIECES:
                nc.vector.tensor_scalar(ov3[:, 0, p0:p1], rv[:, p0:p1],
                                        -1.0, 1.0, Alu.mult, Alu.add)
            # fire the prepared scatter; Tile defers the ov3 read-dep here.
            # The shim republishes DMA completion on the Tile DMASW lane sem
            # (post-compile rewrite) since tlsim only fires on_update[0].
            nc.gpsimd.trigger_dma(count=None)
            nc.gpsimd.wait_ge(ssem, 16).then_inc(shim_sem, 16)

    nc.compile()

    # Post-compile: the end-of-program barrier waits on the Tile-assigned
    # DMASW lane sem, which on HW is bumped by the SWDGE ring itself but in
    # the timeline sim only fires via on_update[0] of the prep (the user
    # sem).  Repoint the shim memset's then_inc at the DMASW sem so both
    # worlds see completion (HW: a second, later bump — harmless for >= 16).
    fn = nc.m.functions[0]
    insts = [i for bb in fn.blocks for i in bb.instructions]
    name_by_id = {}
    for inst in insts:
        si = inst.sync_info
        if not si:
            continue
        for w in si.on_wait:
            name_by_id[w.id] = w.ant_name
        for u in si.on_update:
            name_by_id[u.id] = u.ant_name
    dmasw = sorted((i, n) for i, n in name_by_id.items()
                   if n and n.startswith("DMASW"))
    assert dmasw, "no DMASW lane sem found"
    for inst in insts:
        si = inst.sync_info
        if not si:
            continue
        for u in si.on_update:
            if u.ant_name == "shim_dma":
                u.id = dmasw[0][0]
                u.ant_name = dmasw[0][1]
    return nc


def _get_nc():
    if "nc" not in _cache:
        _cache["nc"] = _build_program()
    return _cache["nc"]


def _shard_inputs(x, weight, in_bf16=IN_BF16):
    w = np.asarray(weight, dtype=np.float32)
    # G rows must be kh-major (kh*32 + c) to match T; cols (kw, f)
    wsc = np.ascontiguousarray(
        w.reshape(C, K, K, F).transpose(1, 0, 2, 3).reshape(96, 96))
    xf = np.asarray(x, dtype=np.float32)
    in_maps = []
    for core in range(NCORES):
        b, half = core // 2, core % 2
        x32 = xf[b, :, 32 * half:32 * half + RIN, :].reshape(C, W32)
        # q-major: partition 32q + c holds x32[c, 561q : 561(q+1)]
        xq = np.ascontiguousarray(
            x32.reshape(C, 4, QW).transpose(1, 0, 2).reshape(128, QW))
        if in_bf16:
            import ml_dtypes
            xq = xq.astype(ml_dtypes.bfloat16)
        import ml_dtypes
        zout = np.zeros((128, 1, OW), dtype=ml_dtypes.bfloat16)
        # SWDGE maps SBUF partition p to idx slot [p % 16, p // 16]
        idx = np.tile(
            np.ascontiguousarray(np.arange(128, dtype=np.int16).reshape(8, 16).T),
            (8, 1))
        in_maps.append({"x_c": xq, "w_sc": wsc, "out": zout, "idx_d": idx})
    return in_maps


def _unshard(results):
    out = np.empty((B, F, S, S), dtype=np.float32)
    for core in range(NCORES):
        b, half = core // 2, core % 2
        res = np.asarray(results[core]["out"],
                         dtype=np.float32).reshape(128, OW)[:, :QW]
        # partition 32g+f, col u -> wide position 561g + u
        wide = res.reshape(4, F, QW).transpose(1, 0, 2).reshape(F, W32)
        blk = wide[:, :32 * H].reshape(F, 32, H)[:, :, :S]
        out[b, :, 32 * half:32 * half + 32, :] = blk
    return out


def kernel(x, weight):
    from concourse.bass_utils import run_bass_kernel_spmd

    nc = _get_nc()
    in_maps = _shard_inputs(x, weight)
    res = run_bass_kernel_spmd(nc, in_maps, list(range(NCORES)))
    return _unshard(res.results)
